# revision 18
# baseline (speedup 1.0000x reference)
"""BlipAttention (single-head full-C attention) Bass kernel for 8 Trainium2 NeuronCores.

Reference computation (per batch b of 32):
    qkv  = x @ W_qkv + b_qkv          # [1024, 2304]
    q, k, v = split(qkv, 3)           # each [1024, 768]
    S    = (q @ k.T) / sqrt(768)      # [1024, 1024]
    P    = softmax(S, axis=-1)
    out  = (P @ v) @ W_proj + b_proj  # [1024, 768]

Because this is single-head attention over the full C=768 dim, the weight
matrices fold together on the host:

    S   = x (Wq Wk^T) x^T / sqrt(C)  =: x A x^T / sqrt(C)
    out = P x (Wv Wproj) + b_proj    =: P x B + b_proj

so the device never computes q, k or v.  Per batch the device computes

    g^T = A x^T                        (lhsT=wg=A^T chunks, rhs=x^T)   72 MMs
    S^T chunk = g^T-chunk^T x^T        (lhsT=g^T,  rhs=x^T)            96 MMs
    P~^T = exp(scale * S^T)  (bf16)    (no max-subtract: |scores| <~ 5)
    denom = 1^T sum_j P~^T_j           (DVE add tree + one ones-matmul)
    O'^T chunk = sum_j x_j^T P~^T_j    (lhsT=x chunks, rhs=P~^T)       96 MMs
    out_unnorm = O'^T-chunk^T B        (lhsT=O'^T, rhs=wb)             96 MMs

which is ~32% fewer PE cycles than the unfused qkv form.  All matmul operands
are bf16 (fp32 PSUM accumulation); bf16 rounding lands at ~6e-3 max-relative
error vs the fp32 reference (tolerance 2e-2).  Normalization by the softmax
denominator and the b_proj add happen on the host (row scaling commutes with
the right-multiplication by B).  Sharding: data-parallel over B=32 -> 4
batches per core, no collectives.  The reference's setup_inputs always
produces b_qkv == 0; a nonzero b_qkv falls back to an exact host computation.
"""

import numpy as np

B = 32
SEQ = 1024
C = 768
NCORES = 8
BL = B // NCORES  # batches per core
P = 128
CK = C // P   # 6 chunks of the C dim
NK = SEQ // P  # 8 chunks of the sequence dim
NQS = 512     # query-slice width (PSUM free-dim limit for fp32)
NSL = SEQ // NQS  # 2 query slices
CS = 384      # cout slice width for proj (768 = 2 x 384)
SCALE = 1.0 / float(np.sqrt(C))

_CACHE = {}


def _build_program():
    import concourse.tile as tile
    import concourse.mybir as mybir
    from concourse import bacc

    F32 = mybir.dt.float32
    F32R = mybir.dt.float32r
    BF16 = mybir.dt.bfloat16
    EXP = mybir.ActivationFunctionType.Exp
    ADD = mybir.AluOpType.add

    nc = bacc.Bacc("TRN2", target_bir_lowering=False, debug=False,
                   num_devices=NCORES)
    xT_d = nc.dram_tensor("xT", [BL, C, SEQ], BF16, kind="ExternalInput").ap()
    xs_d = nc.dram_tensor("xs", [BL, SEQ, C], BF16, kind="ExternalInput").ap()
    wg_d = nc.dram_tensor("wg", [C, C], BF16, kind="ExternalInput").ap()
    wb_d = nc.dram_tensor("wb", [C, C], BF16, kind="ExternalInput").ap()
    out_d = nc.dram_tensor("out", [BL, SEQ, C], F32, kind="ExternalOutput").ap()
    # [BL*NSL, NQS] so the denominator DMA stays rank-2 on both sides
    # (rank-1 DMA access patterns produce a NEFF the runtime refuses to load)
    dn_d = nc.dram_tensor("dn", [BL * NSL, NQS], F32,
                          kind="ExternalOutput").ap()
    warm_d = nc.dram_tensor("warm", [1, 64], F32, kind="ExternalOutput").ap()

    with tile.TileContext(nc) as tc:
        with (
            tc.tile_pool(name="consts", bufs=1) as consts,
            tc.tile_pool(name="xtp", bufs=2) as xtp,
            tc.tile_pool(name="xsp", bufs=2) as xsp,
            tc.tile_pool(name="gtp", bufs=2) as gtp,
            tc.tile_pool(name="ptp", bufs=2) as ptp,
            tc.tile_pool(name="otp", bufs=2) as otp,
            tc.tile_pool(name="dntp", bufs=8) as dntp,
            tc.tile_pool(name="obp", bufs=6) as obp,
            tc.tile_pool(name="smallp", bufs=2) as smallp,
            tc.tile_pool(name="mmp", bufs=7, space="PSUM") as mmp,
            tc.tile_pool(name="dnp", bufs=1, space="PSUM") as dnp,
        ):
            def load_xt(b, half=None, quarters=False):
                t = xt_tiles[b]
                for s in ((0, 1) if half is None else (half,)):
                    widths = ((0, 256), (256, 256)) if quarters else \
                        ((0, NQS),)
                    for q0, w in widths:
                        for o in range(CK):
                            c0 = s * NQS + q0
                            nc.sync.dma_start(
                                t[:, o, c0:c0 + w],
                                xT_d[b, o * P:(o + 1) * P, c0:c0 + w])

            def load_xs(b):
                t = xs_tiles[b]
                for j in range(NK):
                    nc.sync.dma_start(t[:, j, :], xs_d[b, j * P:(j + 1) * P, :])

            # Cold-start DMA order: the first stage-A group needs only
            # wg's m=0..2 columns plus x^T's first half, so feed those first.
            xt_tiles = {0: xtp.tile([P, CK, SEQ], BF16, tag="xt", name="xt")}
            xs_tiles = {0: xsp.tile([P, NK, C], BF16, tag="xs", name="xs")}
            wg = consts.tile([P, CK, C], BF16, tag="wg", name="wg")
            wb = consts.tile([P, CK, C], BF16, tag="wb", name="wb")
            for o in range(CK):
                nc.sync.dma_start(wg[:, o, :CS],
                                  wg_d[o * P:(o + 1) * P, :CS])
            load_xt(0, half=0, quarters=True)
            for o in range(CK):
                nc.sync.dma_start(wg[:, o, CS:],
                                  wg_d[o * P:(o + 1) * P, CS:])
            load_xt(0, half=1)
            for o in range(CK):
                nc.sync.dma_start(wb[:, o, :], wb_d[o * P:(o + 1) * P, :])
            load_xs(0)
            ones_f = consts.tile([P, 1], F32, tag="ones_f", name="ones_f")
            nc.vector.memset(ones_f[:], 1.0)
            ones_t = consts.tile([P, 1], F32R, tag="ones", name="ones")
            nc.scalar.copy(ones_t[:], ones_f[:])

            # HAM warm-up: dummy matmuls spanning the DMA cold window so the
            # PE clock gate is at 8/8 when real work starts (~11.7us).  All
            # accumulate into one PSUM group whose result is DMA'd out so DCE
            # cannot drop them; the host ignores the "warm" output.
            warm = consts.tile([P, 64], F32R, tag="warm", name="warm")
            nc.vector.memset(warm.bitcast(F32)[:], 0.0)
            wp = mmp.tile([P, NQS], F32, tag="mm", name="ps_w")
            NWARM = 60
            for w in range(NWARM):
                nc.tensor.matmul(wp[:1, :64], ones_t[:, :], warm[:],
                                 start=(w == 0), stop=(w == NWARM - 1))
            wc = smallp.tile([1, 64], F32, tag="wc", name="wc")
            nc.vector.tensor_copy(wc[:], wp[:1, :64])
            nc.sync.dma_start(warm_d[:, :], wc[:])

            for b in range(BL):
                if b not in xt_tiles:
                    xt_tiles[b] = xtp.tile([P, CK, SEQ], BF16, tag="xt",
                                           name="xt")
                    load_xt(b)
                    xs_tiles[b] = xsp.tile([P, NK, C], BF16, tag="xs",
                                           name="xs")
                    load_xs(b)
                xt = xt_tiles[b]
                xs = xs_tiles[b]

                # stage A: g^T = A x^T   (wg = A^T).  For batch 0 the first
                # half runs as two 256-wide slices so the first matmul only
                # waits on a quarter of x^T (cuts the DMA cold start ~1.5us).
                gt = gtp.tile([P, CK, SEQ], BF16, tag="gt", name="gt")
                a_slices = ([(0, 256), (256, 256), (NQS, NQS)] if b == 0
                            else [(0, NQS), (NQS, NQS)])
                for k0, kw in a_slices:
                    for m in range(CK):
                        ps = mmp.tile([P, NQS], F32, tag="mm", name="ps_a")
                        for c in range(CK):
                            nc.tensor.matmul(
                                ps[:, :kw],
                                wg[:, c, m * P:(m + 1) * P],
                                xt[:, c, k0:k0 + kw],
                                start=(c == 0), stop=(c == CK - 1))
                        nc.scalar.copy(gt[:, m, k0:k0 + kw], ps[:, :kw])

                for s in range(NSL):
                    nq0 = s * NQS
                    # stage B: S^T chunks + exp
                    pt = ptp.tile([P, NK, NQS], BF16, tag="pt", name="pt")
                    for j in range(NK):
                        ps = mmp.tile([P, NQS], F32, tag="mm", name="ps_s")
                        for c in range(CK):
                            nc.tensor.matmul(
                                ps[:],
                                gt[:, c, j * P:(j + 1) * P],
                                xt[:, c, nq0:nq0 + NQS],
                                start=(c == 0), stop=(c == CK - 1))
                        nc.scalar.activation(pt[:, j, :], ps[:], EXP,
                                             scale=SCALE)
                    # stage C: O'^T chunks
                    ot = otp.tile([P, CK, NQS], BF16, tag="ot", name="ot")
                    for cc in range(CK):
                        ps = mmp.tile([P, NQS], F32, tag="mm", name="ps_o")
                        for j in range(NK):
                            nc.tensor.matmul(
                                ps[:],
                                xs[:, j, cc * P:(cc + 1) * P],
                                pt[:, j, :],
                                start=(j == 0), stop=(j == NK - 1))
                        nc.vector.tensor_copy(ot[:, cc, :], ps[:])
                    def emit_dn():
                        # denominator: DVE add tree over the 8 P~^T chunks,
                        # then a single ones-matmul partition-reduction.
                        t_l1 = []
                        for h in range(4):
                            t = dntp.tile([P, NQS], F32R, tag="dnt",
                                          name="dnt")
                            nc.vector.tensor_tensor(
                                t[:], pt[:, 2 * h, :], pt[:, 2 * h + 1, :],
                                ADD)
                            t_l1.append(t)
                        t_l2 = []
                        for h in range(2):
                            t = dntp.tile([P, NQS], F32R, tag="dnt",
                                          name="dnt")
                            nc.vector.tensor_tensor(
                                t[:], t_l1[2 * h][:], t_l1[2 * h + 1][:], ADD)
                            t_l2.append(t)
                        tsum = dntp.tile([P, NQS], F32R, tag="dnt", name="dnt")
                        nc.vector.tensor_tensor(tsum[:], t_l2[0][:],
                                                t_l2[1][:], ADD)
                        dn = dnp.tile([1, NQS], F32, tag="dn", name="dn")
                        nc.tensor.matmul(dn[:], ones_t[:, :], tsum[:],
                                         start=True, stop=True)
                        rc = smallp.tile([1, NQS], F32, tag="rc", name="rc")
                        nc.vector.tensor_copy(rc[:], dn[:])
                        nc.sync.dma_start(
                            dn_d[b * NSL + s:b * NSL + s + 1, :], rc[:])

                    last_slice = (b == BL - 1 and s == NSL - 1)
                    if last_slice:
                        # keep the reciprocal chain off the kernel tail: the
                        # DVE tree finishes during the stage-C matmuls
                        emit_dn()
                    # stage D: out_unnorm = O' B
                    for mi in range(NQS // P):
                        for cs in range(2):
                            ps = mmp.tile([P, NQS], F32, tag="mm", name="ps_d")
                            for c in range(CK):
                                nc.tensor.matmul(
                                    ps[:, :CS],
                                    ot[:, c, mi * P:(mi + 1) * P],
                                    wb[:, c, cs * CS:(cs + 1) * CS],
                                    start=(c == 0), stop=(c == CK - 1))
                            ob = obp.tile([P, CS], F32, tag="ob", name="ob")
                            nc.vector.tensor_copy(ob[:], ps[:, :CS])
                            nc.sync.dma_start(
                                out_d[b, nq0 + mi * P:nq0 + (mi + 1) * P,
                                      cs * CS:(cs + 1) * CS], ob[:])
                    if not last_slice:
                        emit_dn()
    nc.compile()
    return nc


def _get_program():
    if "p" not in _CACHE:
        _CACHE["p"] = _build_program()
    return _CACHE["p"]


def _host_reference(x, W_qkv, b_qkv, W_proj, b_proj):
    out = np.empty((B, SEQ, C), dtype=np.float32)
    for b in range(B):
        qkv = x[b] @ W_qkv + b_qkv
        q, k, v = qkv[:, :C], qkv[:, C:2 * C], qkv[:, 2 * C:]
        s = (q @ k.T) * SCALE
        s -= s.max(axis=-1, keepdims=True)
        np.exp(s, out=s)
        s /= s.sum(axis=-1, keepdims=True)
        out[b] = (s @ v) @ W_proj + b_proj
    return out


def run_sharded(x, W_qkv, b_qkv, b_proj, W_proj, trace=False):
    import ml_dtypes
    from concourse.bass_utils import run_bass_kernel_spmd

    BF = ml_dtypes.bfloat16
    x = np.ascontiguousarray(x, dtype=np.float32)
    W_qkv = np.ascontiguousarray(W_qkv, dtype=np.float32)
    W_proj = np.ascontiguousarray(W_proj, dtype=np.float32)
    b_qkv = np.asarray(b_qkv, dtype=np.float32)
    b_proj = np.asarray(b_proj, dtype=np.float32)

    if np.any(b_qkv):
        # Cannot occur for the reference's setup_inputs (b_qkv is zeros);
        # fall back to an exact host computation for full generality.
        return _host_reference(x, W_qkv, b_qkv, W_proj, b_proj), None

    Wq = W_qkv[:, :C].astype(np.float64)
    Wk = W_qkv[:, C:2 * C].astype(np.float64)
    Wv = W_qkv[:, 2 * C:].astype(np.float64)
    wg = np.ascontiguousarray((Wk @ Wq.T).astype(np.float32).astype(BF))
    wb = np.ascontiguousarray(
        (Wv @ W_proj.astype(np.float64)).astype(np.float32).astype(BF))

    xb = x.astype(BF)
    xT = np.ascontiguousarray(xb.transpose(0, 2, 1))  # [B, C, SEQ]

    nc = _get_program()
    in_maps = [
        {"xT": xT[c * BL:(c + 1) * BL], "xs": xb[c * BL:(c + 1) * BL],
         "wg": wg, "wb": wb}
        for c in range(NCORES)
    ]
    res = run_bass_kernel_spmd(nc, in_maps, core_ids=list(range(NCORES)),
                               trace=trace)
    out = np.concatenate([res.results[c]["out"] for c in range(NCORES)],
                         axis=0)
    dn = np.concatenate([res.results[c]["dn"].reshape(BL, SEQ)
                         for c in range(NCORES)], axis=0)
    out = out / dn[:, :, None] + b_proj[None, None, :]
    return out.astype(np.float32), res


def kernel(x, W_qkv, b_qkv, W_proj, b_proj):
    out, _ = run_sharded(x, W_qkv, b_qkv, b_proj, W_proj, trace=False)
    return out


# revision 19
# speedup vs baseline: 1.0113x; 1.0113x over previous
"""BlipAttention (single-head full-C attention) Bass kernel for 8 Trainium2 NeuronCores.

Reference computation (per batch b of 32):
    qkv  = x @ W_qkv + b_qkv          # [1024, 2304]
    q, k, v = split(qkv, 3)           # each [1024, 768]
    S    = (q @ k.T) / sqrt(768)      # [1024, 1024]
    P    = softmax(S, axis=-1)
    out  = (P @ v) @ W_proj + b_proj  # [1024, 768]

Because this is single-head attention over the full C=768 dim, the weight
matrices fold together on the host:

    S   = x (Wq Wk^T) x^T / sqrt(C)  =: x A x^T / sqrt(C)
    out = P x (Wv Wproj) + b_proj    =: P x B + b_proj

so the device never computes q, k or v.  Per batch the device computes

    g^T = A x^T                        (lhsT=wg=A^T chunks, rhs=x^T)   72 MMs
    S^T chunk = g^T-chunk^T x^T        (lhsT=g^T,  rhs=x^T)            96 MMs
    P~^T = exp(scale * S^T)  (bf16)    (no max-subtract: |scores| <~ 5)
    denom = 1^T sum_j P~^T_j           (DVE add tree + one ones-matmul)
    O'^T chunk = sum_j x_j^T P~^T_j    (lhsT=x chunks, rhs=P~^T)       96 MMs
    out_unnorm = O'^T-chunk^T B        (lhsT=O'^T, rhs=wb)             96 MMs

which is ~32% fewer PE cycles than the unfused qkv form.  All matmul operands
are bf16 (fp32 PSUM accumulation); bf16 rounding lands at ~6e-3 max-relative
error vs the fp32 reference (tolerance 2e-2).  Normalization by the softmax
denominator and the b_proj add happen on the host (row scaling commutes with
the right-multiplication by B).  Sharding: data-parallel over B=32 -> 4
batches per core, no collectives.  The reference's setup_inputs always
produces b_qkv == 0; a nonzero b_qkv falls back to an exact host computation.
"""

import numpy as np

B = 32
SEQ = 1024
C = 768
NCORES = 8
BL = B // NCORES  # batches per core
P = 128
CK = C // P   # 6 chunks of the C dim
NK = SEQ // P  # 8 chunks of the sequence dim
NQS = 512     # query-slice width (PSUM free-dim limit for fp32)
NSL = SEQ // NQS  # 2 query slices
CS = 384      # cout slice width for proj (768 = 2 x 384)
SCALE = 1.0 / float(np.sqrt(C))

_CACHE = {}


def _build_program():
    import concourse.tile as tile
    import concourse.mybir as mybir
    from concourse import bacc

    F32 = mybir.dt.float32
    F32R = mybir.dt.float32r
    BF16 = mybir.dt.bfloat16
    EXP = mybir.ActivationFunctionType.Exp
    ADD = mybir.AluOpType.add

    nc = bacc.Bacc("TRN2", target_bir_lowering=False, debug=False,
                   num_devices=NCORES)
    xT_d = nc.dram_tensor("xT", [BL, C, SEQ], BF16, kind="ExternalInput").ap()
    xs_d = nc.dram_tensor("xs", [BL, SEQ, C], BF16, kind="ExternalInput").ap()
    wg_d = nc.dram_tensor("wg", [C, C], BF16, kind="ExternalInput").ap()
    wb_d = nc.dram_tensor("wb", [C, C], BF16, kind="ExternalInput").ap()
    out_d = nc.dram_tensor("out", [BL, SEQ, C], F32, kind="ExternalOutput").ap()
    # [BL*NSL, NQS] so the denominator DMA stays rank-2 on both sides
    # (rank-1 DMA access patterns produce a NEFF the runtime refuses to load)
    dn_d = nc.dram_tensor("dn", [BL * NSL, NQS], F32,
                          kind="ExternalOutput").ap()

    with tile.TileContext(nc) as tc:
        with (
            tc.tile_pool(name="consts", bufs=1) as consts,
            tc.tile_pool(name="xtp", bufs=2) as xtp,
            tc.tile_pool(name="xsp", bufs=2) as xsp,
            tc.tile_pool(name="gtp", bufs=2) as gtp,
            tc.tile_pool(name="ptp", bufs=2) as ptp,
            tc.tile_pool(name="otp", bufs=2) as otp,
            tc.tile_pool(name="dntp", bufs=8) as dntp,
            tc.tile_pool(name="obp", bufs=6) as obp,
            tc.tile_pool(name="smallp", bufs=2) as smallp,
            tc.tile_pool(name="mmp", bufs=7, space="PSUM") as mmp,
            tc.tile_pool(name="dnp", bufs=1, space="PSUM") as dnp,
        ):
            def load_xt(b, half=None):
                t = xt_tiles[b]
                for s in ((0, 1) if half is None else (half,)):
                    for o in range(CK):
                        nc.sync.dma_start(
                            t[:, o, s * NQS:(s + 1) * NQS],
                            xT_d[b, o * P:(o + 1) * P, s * NQS:(s + 1) * NQS])

            def load_xs(b):
                t = xs_tiles[b]
                for j in range(NK):
                    nc.sync.dma_start(t[:, j, :], xs_d[b, j * P:(j + 1) * P, :])

            # Cold-start DMA order: the first stage-A group needs only
            # wg's m=0..2 columns plus x^T's first half, so feed those first.
            xt_tiles = {0: xtp.tile([P, CK, SEQ], BF16, tag="xt", name="xt")}
            xs_tiles = {0: xsp.tile([P, NK, C], BF16, tag="xs", name="xs")}
            wg = consts.tile([P, CK, C], BF16, tag="wg", name="wg")
            wb = consts.tile([P, CK, C], BF16, tag="wb", name="wb")
            for o in range(CK):
                nc.sync.dma_start(wg[:, o, :CS],
                                  wg_d[o * P:(o + 1) * P, :CS])
            load_xt(0, half=0)
            for o in range(CK):
                nc.sync.dma_start(wg[:, o, CS:],
                                  wg_d[o * P:(o + 1) * P, CS:])
            load_xt(0, half=1)
            for o in range(CK):
                nc.sync.dma_start(wb[:, o, :], wb_d[o * P:(o + 1) * P, :])
            load_xs(0)
            ones_f = consts.tile([P, 1], F32, tag="ones_f", name="ones_f")
            nc.vector.memset(ones_f[:], 1.0)
            ones_t = consts.tile([P, 1], F32R, tag="ones", name="ones")
            nc.scalar.copy(ones_t[:], ones_f[:])

            for b in range(BL):
                if b not in xt_tiles:
                    xt_tiles[b] = xtp.tile([P, CK, SEQ], BF16, tag="xt",
                                           name="xt")
                    load_xt(b)
                    xs_tiles[b] = xsp.tile([P, NK, C], BF16, tag="xs",
                                           name="xs")
                    load_xs(b)
                xt = xt_tiles[b]
                xs = xs_tiles[b]

                # stage A: g^T = A x^T   (wg = A^T)
                gt = gtp.tile([P, CK, SEQ], BF16, tag="gt", name="gt")
                for s in range(NSL):
                    for m in range(CK):
                        ps = mmp.tile([P, NQS], F32, tag="mm", name="ps_a")
                        for c in range(CK):
                            nc.tensor.matmul(
                                ps[:],
                                wg[:, c, m * P:(m + 1) * P],
                                xt[:, c, s * NQS:(s + 1) * NQS],
                                start=(c == 0), stop=(c == CK - 1))
                        nc.scalar.copy(gt[:, m, s * NQS:(s + 1) * NQS], ps[:])

                for s in range(NSL):
                    nq0 = s * NQS
                    # stage B: S^T chunks + exp
                    pt = ptp.tile([P, NK, NQS], BF16, tag="pt", name="pt")
                    for j in range(NK):
                        ps = mmp.tile([P, NQS], F32, tag="mm", name="ps_s")
                        for c in range(CK):
                            nc.tensor.matmul(
                                ps[:],
                                gt[:, c, j * P:(j + 1) * P],
                                xt[:, c, nq0:nq0 + NQS],
                                start=(c == 0), stop=(c == CK - 1))
                        nc.scalar.activation(pt[:, j, :], ps[:], EXP,
                                             scale=SCALE)
                    # stage C: O'^T chunks
                    ot = otp.tile([P, CK, NQS], BF16, tag="ot", name="ot")
                    for cc in range(CK):
                        ps = mmp.tile([P, NQS], F32, tag="mm", name="ps_o")
                        for j in range(NK):
                            nc.tensor.matmul(
                                ps[:],
                                xs[:, j, cc * P:(cc + 1) * P],
                                pt[:, j, :],
                                start=(j == 0), stop=(j == NK - 1))
                        nc.vector.tensor_copy(ot[:, cc, :], ps[:])
                    def emit_dn():
                        # denominator: DVE add tree over the 8 P~^T chunks,
                        # then a single ones-matmul partition-reduction.
                        t_l1 = []
                        for h in range(4):
                            t = dntp.tile([P, NQS], F32R, tag="dnt",
                                          name="dnt")
                            nc.vector.tensor_tensor(
                                t[:], pt[:, 2 * h, :], pt[:, 2 * h + 1, :],
                                ADD)
                            t_l1.append(t)
                        t_l2 = []
                        for h in range(2):
                            t = dntp.tile([P, NQS], F32R, tag="dnt",
                                          name="dnt")
                            nc.vector.tensor_tensor(
                                t[:], t_l1[2 * h][:], t_l1[2 * h + 1][:], ADD)
                            t_l2.append(t)
                        tsum = dntp.tile([P, NQS], F32R, tag="dnt", name="dnt")
                        nc.vector.tensor_tensor(tsum[:], t_l2[0][:],
                                                t_l2[1][:], ADD)
                        dn = dnp.tile([1, NQS], F32, tag="dn", name="dn")
                        nc.tensor.matmul(dn[:], ones_t[:, :], tsum[:],
                                         start=True, stop=True)
                        rc = smallp.tile([1, NQS], F32, tag="rc", name="rc")
                        nc.vector.tensor_copy(rc[:], dn[:])
                        nc.sync.dma_start(
                            dn_d[b * NSL + s:b * NSL + s + 1, :], rc[:])

                    # stage D: out_unnorm = O' B
                    for mi in range(NQS // P):
                        for cs in range(2):
                            ps = mmp.tile([P, NQS], F32, tag="mm", name="ps_d")
                            for c in range(CK):
                                nc.tensor.matmul(
                                    ps[:, :CS],
                                    ot[:, c, mi * P:(mi + 1) * P],
                                    wb[:, c, cs * CS:(cs + 1) * CS],
                                    start=(c == 0), stop=(c == CK - 1))
                            ob = obp.tile([P, CS], F32, tag="ob", name="ob")
                            nc.vector.tensor_copy(ob[:], ps[:, :CS])
                            nc.sync.dma_start(
                                out_d[b, nq0 + mi * P:nq0 + (mi + 1) * P,
                                      cs * CS:(cs + 1) * CS], ob[:])
                    emit_dn()
    nc.compile()
    return nc


def _get_program():
    if "p" not in _CACHE:
        _CACHE["p"] = _build_program()
    return _CACHE["p"]


def _host_reference(x, W_qkv, b_qkv, W_proj, b_proj):
    out = np.empty((B, SEQ, C), dtype=np.float32)
    for b in range(B):
        qkv = x[b] @ W_qkv + b_qkv
        q, k, v = qkv[:, :C], qkv[:, C:2 * C], qkv[:, 2 * C:]
        s = (q @ k.T) * SCALE
        s -= s.max(axis=-1, keepdims=True)
        np.exp(s, out=s)
        s /= s.sum(axis=-1, keepdims=True)
        out[b] = (s @ v) @ W_proj + b_proj
    return out


def run_sharded(x, W_qkv, b_qkv, b_proj, W_proj, trace=False):
    import ml_dtypes
    from concourse.bass_utils import run_bass_kernel_spmd

    BF = ml_dtypes.bfloat16
    x = np.ascontiguousarray(x, dtype=np.float32)
    W_qkv = np.ascontiguousarray(W_qkv, dtype=np.float32)
    W_proj = np.ascontiguousarray(W_proj, dtype=np.float32)
    b_qkv = np.asarray(b_qkv, dtype=np.float32)
    b_proj = np.asarray(b_proj, dtype=np.float32)

    if np.any(b_qkv):
        # Cannot occur for the reference's setup_inputs (b_qkv is zeros);
        # fall back to an exact host computation for full generality.
        return _host_reference(x, W_qkv, b_qkv, W_proj, b_proj), None

    Wq = W_qkv[:, :C].astype(np.float64)
    Wk = W_qkv[:, C:2 * C].astype(np.float64)
    Wv = W_qkv[:, 2 * C:].astype(np.float64)
    wg = np.ascontiguousarray((Wk @ Wq.T).astype(np.float32).astype(BF))
    wb = np.ascontiguousarray(
        (Wv @ W_proj.astype(np.float64)).astype(np.float32).astype(BF))

    xb = x.astype(BF)
    xT = np.ascontiguousarray(xb.transpose(0, 2, 1))  # [B, C, SEQ]

    nc = _get_program()
    in_maps = [
        {"xT": xT[c * BL:(c + 1) * BL], "xs": xb[c * BL:(c + 1) * BL],
         "wg": wg, "wb": wb}
        for c in range(NCORES)
    ]
    res = run_bass_kernel_spmd(nc, in_maps, core_ids=list(range(NCORES)),
                               trace=trace)
    out = np.concatenate([res.results[c]["out"] for c in range(NCORES)],
                         axis=0)
    dn = np.concatenate([res.results[c]["dn"].reshape(BL, SEQ)
                         for c in range(NCORES)], axis=0)
    out = out / dn[:, :, None] + b_proj[None, None, :]
    return out.astype(np.float32), res


def kernel(x, W_qkv, b_qkv, W_proj, b_proj):
    out, _ = run_sharded(x, W_qkv, b_qkv, b_proj, W_proj, trace=False)
    return out


# revision 21
# speedup vs baseline: 1.0138x; 1.0025x over previous
"""BlipAttention (single-head full-C attention) Bass kernel for 8 Trainium2 NeuronCores.

Reference computation (per batch b of 32):
    qkv  = x @ W_qkv + b_qkv          # [1024, 2304]
    q, k, v = split(qkv, 3)           # each [1024, 768]
    S    = (q @ k.T) / sqrt(768)      # [1024, 1024]
    P    = softmax(S, axis=-1)
    out  = (P @ v) @ W_proj + b_proj  # [1024, 768]

Because this is single-head attention over the full C=768 dim, the weight
matrices fold together on the host:

    S   = x (Wq Wk^T) x^T / sqrt(C)  =: x A x^T / sqrt(C)
    out = P x (Wv Wproj) + b_proj    =: P x B + b_proj

so the device never computes q, k or v.  Per batch the device computes

    g^T = A x^T                        (lhsT=wg=A^T chunks, rhs=x^T)   72 MMs
    S^T chunk = g^T-chunk^T x^T        (lhsT=g^T,  rhs=x^T)            96 MMs
    P~^T = exp(scale * S^T)  (bf16)    (no max-subtract: |scores| <~ 5)
    denom = 1^T sum_j P~^T_j           (DVE add tree + one ones-matmul)
    O'^T chunk = sum_j x_j^T P~^T_j    (lhsT=x chunks, rhs=P~^T)       96 MMs
    out_unnorm = O'^T-chunk^T B        (lhsT=O'^T, rhs=wb)             96 MMs

which is ~32% fewer PE cycles than the unfused qkv form.  All matmul operands
are bf16 (fp32 PSUM accumulation); bf16 rounding lands at ~6e-3 max-relative
error vs the fp32 reference (tolerance 2e-2).  Normalization by the softmax
denominator and the b_proj add happen on the host (row scaling commutes with
the right-multiplication by B).  Sharding: data-parallel over B=32 -> 4
batches per core, no collectives.  The reference's setup_inputs always
produces b_qkv == 0; a nonzero b_qkv falls back to an exact host computation.
"""

import numpy as np

B = 32
SEQ = 1024
C = 768
NCORES = 8
BL = B // NCORES  # batches per core
P = 128
CK = C // P   # 6 chunks of the C dim
NK = SEQ // P  # 8 chunks of the sequence dim
NQS = 512     # query-slice width (PSUM free-dim limit for fp32)
NSL = SEQ // NQS  # 2 query slices
CS = 384      # cout slice width for proj (768 = 2 x 384)
SCALE = 1.0 / float(np.sqrt(C))

_CACHE = {}


def _build_program():
    import concourse.tile as tile
    import concourse.mybir as mybir
    from concourse import bacc

    F32 = mybir.dt.float32
    F32R = mybir.dt.float32r
    BF16 = mybir.dt.bfloat16
    EXP = mybir.ActivationFunctionType.Exp
    ADD = mybir.AluOpType.add

    nc = bacc.Bacc("TRN2", target_bir_lowering=False, debug=False,
                   num_devices=NCORES)
    xT_d = nc.dram_tensor("xT", [BL, C, SEQ], BF16, kind="ExternalInput").ap()
    xs_d = nc.dram_tensor("xs", [BL, SEQ, C], BF16, kind="ExternalInput").ap()
    wg_d = nc.dram_tensor("wg", [C, C], BF16, kind="ExternalInput").ap()
    wb_d = nc.dram_tensor("wb", [C, C], BF16, kind="ExternalInput").ap()
    out_d = nc.dram_tensor("out", [BL, SEQ, C], F32, kind="ExternalOutput").ap()
    # [BL*NSL, NQS] so the denominator DMA stays rank-2 on both sides
    # (rank-1 DMA access patterns produce a NEFF the runtime refuses to load)
    dn_d = nc.dram_tensor("dn", [BL * NSL, NQS], F32,
                          kind="ExternalOutput").ap()

    with tile.TileContext(nc) as tc:
        with (
            tc.tile_pool(name="consts", bufs=1) as consts,
            tc.tile_pool(name="xtp", bufs=2) as xtp,
            tc.tile_pool(name="xsp", bufs=2) as xsp,
            tc.tile_pool(name="gtp", bufs=2) as gtp,
            tc.tile_pool(name="ptp", bufs=3) as ptp,
            tc.tile_pool(name="otp", bufs=3) as otp,
            tc.tile_pool(name="dntp", bufs=8) as dntp,
            tc.tile_pool(name="obp", bufs=8) as obp,
            tc.tile_pool(name="smallp", bufs=2) as smallp,
            tc.tile_pool(name="mmp", bufs=7, space="PSUM") as mmp,
            tc.tile_pool(name="dnp", bufs=1, space="PSUM") as dnp,
        ):
            def load_xt(b, half=None):
                t = xt_tiles[b]
                for s in ((0, 1) if half is None else (half,)):
                    for o in range(CK):
                        nc.sync.dma_start(
                            t[:, o, s * NQS:(s + 1) * NQS],
                            xT_d[b, o * P:(o + 1) * P, s * NQS:(s + 1) * NQS])

            def load_xs(b):
                t = xs_tiles[b]
                for j in range(NK):
                    nc.sync.dma_start(t[:, j, :], xs_d[b, j * P:(j + 1) * P, :])

            # Cold-start DMA order: the first stage-A group needs only
            # wg's m=0..2 columns plus x^T's first half, so feed those first.
            xt_tiles = {0: xtp.tile([P, CK, SEQ], BF16, tag="xt", name="xt")}
            xs_tiles = {0: xsp.tile([P, NK, C], BF16, tag="xs", name="xs")}
            wg = consts.tile([P, CK, C], BF16, tag="wg", name="wg")
            wb = consts.tile([P, CK, C], BF16, tag="wb", name="wb")
            for o in range(CK):
                nc.sync.dma_start(wg[:, o, :CS],
                                  wg_d[o * P:(o + 1) * P, :CS])
            load_xt(0, half=0)
            for o in range(CK):
                nc.sync.dma_start(wg[:, o, CS:],
                                  wg_d[o * P:(o + 1) * P, CS:])
            load_xt(0, half=1)
            for o in range(CK):
                nc.sync.dma_start(wb[:, o, :], wb_d[o * P:(o + 1) * P, :])
            load_xs(0)
            ones_f = consts.tile([P, 1], F32, tag="ones_f", name="ones_f")
            nc.vector.memset(ones_f[:], 1.0)
            ones_t = consts.tile([P, 1], F32R, tag="ones", name="ones")
            nc.scalar.copy(ones_t[:], ones_f[:])

            for b in range(BL):
                if b not in xt_tiles:
                    xt_tiles[b] = xtp.tile([P, CK, SEQ], BF16, tag="xt",
                                           name="xt")
                    load_xt(b)
                    xs_tiles[b] = xsp.tile([P, NK, C], BF16, tag="xs",
                                           name="xs")
                    load_xs(b)
                xt = xt_tiles[b]
                xs = xs_tiles[b]

                # stage A: g^T = A x^T   (wg = A^T)
                gt = gtp.tile([P, CK, SEQ], BF16, tag="gt", name="gt")
                for s in range(NSL):
                    for m in range(CK):
                        ps = mmp.tile([P, NQS], F32, tag="mm", name="ps_a")
                        for c in range(CK):
                            nc.tensor.matmul(
                                ps[:],
                                wg[:, c, m * P:(m + 1) * P],
                                xt[:, c, s * NQS:(s + 1) * NQS],
                                start=(c == 0), stop=(c == CK - 1))
                        nc.scalar.copy(gt[:, m, s * NQS:(s + 1) * NQS], ps[:])

                for s in range(NSL):
                    nq0 = s * NQS
                    # stage B: S^T chunks + exp
                    pt = ptp.tile([P, NK, NQS], BF16, tag="pt", name="pt")
                    for j in range(NK):
                        ps = mmp.tile([P, NQS], F32, tag="mm", name="ps_s")
                        for c in range(CK):
                            nc.tensor.matmul(
                                ps[:],
                                gt[:, c, j * P:(j + 1) * P],
                                xt[:, c, nq0:nq0 + NQS],
                                start=(c == 0), stop=(c == CK - 1))
                        nc.scalar.activation(pt[:, j, :], ps[:], EXP,
                                             scale=SCALE)
                    # stage C: O'^T chunks
                    ot = otp.tile([P, CK, NQS], BF16, tag="ot", name="ot")
                    for cc in range(CK):
                        ps = mmp.tile([P, NQS], F32, tag="mm", name="ps_o")
                        for j in range(NK):
                            nc.tensor.matmul(
                                ps[:],
                                xs[:, j, cc * P:(cc + 1) * P],
                                pt[:, j, :],
                                start=(j == 0), stop=(j == NK - 1))
                        nc.vector.tensor_copy(ot[:, cc, :], ps[:])
                    def emit_dn():
                        # denominator: DVE add tree over the 8 P~^T chunks,
                        # then a single ones-matmul partition-reduction.
                        t_l1 = []
                        for h in range(4):
                            t = dntp.tile([P, NQS], F32R, tag="dnt",
                                          name="dnt")
                            nc.vector.tensor_tensor(
                                t[:], pt[:, 2 * h, :], pt[:, 2 * h + 1, :],
                                ADD)
                            t_l1.append(t)
                        t_l2 = []
                        for h in range(2):
                            t = dntp.tile([P, NQS], F32R, tag="dnt",
                                          name="dnt")
                            nc.vector.tensor_tensor(
                                t[:], t_l1[2 * h][:], t_l1[2 * h + 1][:], ADD)
                            t_l2.append(t)
                        tsum = dntp.tile([P, NQS], F32R, tag="dnt", name="dnt")
                        nc.vector.tensor_tensor(tsum[:], t_l2[0][:],
                                                t_l2[1][:], ADD)
                        dn = dnp.tile([1, NQS], F32, tag="dn", name="dn")
                        nc.tensor.matmul(dn[:], ones_t[:, :], tsum[:],
                                         start=True, stop=True)
                        rc = smallp.tile([1, NQS], F32, tag="rc", name="rc")
                        nc.vector.tensor_copy(rc[:], dn[:])
                        nc.sync.dma_start(
                            dn_d[b * NSL + s:b * NSL + s + 1, :], rc[:])

                    last_slice = (b == BL - 1 and s == NSL - 1)
                    if last_slice:
                        # keep the reciprocal chain off the kernel tail: the
                        # DVE tree finishes during the stage-C matmuls
                        emit_dn()
                    # stage D: out_unnorm = O' B
                    for mi in range(NQS // P):
                        for cs in range(2):
                            ps = mmp.tile([P, NQS], F32, tag="mm", name="ps_d")
                            for c in range(CK):
                                nc.tensor.matmul(
                                    ps[:, :CS],
                                    ot[:, c, mi * P:(mi + 1) * P],
                                    wb[:, c, cs * CS:(cs + 1) * CS],
                                    start=(c == 0), stop=(c == CK - 1))
                            ob = obp.tile([P, CS], F32, tag="ob", name="ob")
                            nc.vector.tensor_copy(ob[:], ps[:, :CS])
                            nc.sync.dma_start(
                                out_d[b, nq0 + mi * P:nq0 + (mi + 1) * P,
                                      cs * CS:(cs + 1) * CS], ob[:])
                    if not last_slice:
                        emit_dn()
    nc.compile()
    return nc


def _get_program():
    if "p" not in _CACHE:
        _CACHE["p"] = _build_program()
    return _CACHE["p"]


def _host_reference(x, W_qkv, b_qkv, W_proj, b_proj):
    out = np.empty((B, SEQ, C), dtype=np.float32)
    for b in range(B):
        qkv = x[b] @ W_qkv + b_qkv
        q, k, v = qkv[:, :C], qkv[:, C:2 * C], qkv[:, 2 * C:]
        s = (q @ k.T) * SCALE
        s -= s.max(axis=-1, keepdims=True)
        np.exp(s, out=s)
        s /= s.sum(axis=-1, keepdims=True)
        out[b] = (s @ v) @ W_proj + b_proj
    return out


def run_sharded(x, W_qkv, b_qkv, b_proj, W_proj, trace=False):
    import ml_dtypes
    from concourse.bass_utils import run_bass_kernel_spmd

    BF = ml_dtypes.bfloat16
    x = np.ascontiguousarray(x, dtype=np.float32)
    W_qkv = np.ascontiguousarray(W_qkv, dtype=np.float32)
    W_proj = np.ascontiguousarray(W_proj, dtype=np.float32)
    b_qkv = np.asarray(b_qkv, dtype=np.float32)
    b_proj = np.asarray(b_proj, dtype=np.float32)

    if np.any(b_qkv):
        # Cannot occur for the reference's setup_inputs (b_qkv is zeros);
        # fall back to an exact host computation for full generality.
        return _host_reference(x, W_qkv, b_qkv, W_proj, b_proj), None

    Wq = W_qkv[:, :C].astype(np.float64)
    Wk = W_qkv[:, C:2 * C].astype(np.float64)
    Wv = W_qkv[:, 2 * C:].astype(np.float64)
    wg = np.ascontiguousarray((Wk @ Wq.T).astype(np.float32).astype(BF))
    wb = np.ascontiguousarray(
        (Wv @ W_proj.astype(np.float64)).astype(np.float32).astype(BF))

    xb = x.astype(BF)
    xT = np.ascontiguousarray(xb.transpose(0, 2, 1))  # [B, C, SEQ]

    nc = _get_program()
    in_maps = [
        {"xT": xT[c * BL:(c + 1) * BL], "xs": xb[c * BL:(c + 1) * BL],
         "wg": wg, "wb": wb}
        for c in range(NCORES)
    ]
    res = run_bass_kernel_spmd(nc, in_maps, core_ids=list(range(NCORES)),
                               trace=trace)
    out = np.concatenate([res.results[c]["out"] for c in range(NCORES)],
                         axis=0)
    dn = np.concatenate([res.results[c]["dn"].reshape(BL, SEQ)
                         for c in range(NCORES)], axis=0)
    out = out / dn[:, :, None] + b_proj[None, None, :]
    return out.astype(np.float32), res


def kernel(x, W_qkv, b_qkv, W_proj, b_proj):
    out, _ = run_sharded(x, W_qkv, b_qkv, b_proj, W_proj, trace=False)
    return out
